# revision 16
# baseline (speedup 1.0000x reference)
"""Trainium2 Bass kernel for a 10-layer tiny MLP over 8.4M 2-D points.

reference: h <- tanh(h @ W_l^T + b_l) for 10 layers (2->2), then y = h @ W_out^T + b_out.

Strategy (8 NeuronCores, pure data parallel over the batch):
  - Each core gets 1,048,576 points, processed as 8 macro-tiles of
    131,072 points ([128 partitions x 2048 f32] SBUF tiles).
  - Layout trick: a DVE 32x32 stream-transpose of the contiguously-loaded
    raw tile puts the two channels of each point on adjacent partitions
    (2s, 2s+1) of the same column (in a bit-shuffled point order, which is
    fine since the whole network is pointwise).
  - Each layer's affine transform is ONE 128x128 matmul per 512-col chunk
    with a block-diagonal weight matrix (64 copies of W_l^T on the
    diagonal), accumulated in PSUM; ScalarE applies bias+tanh in a single
    fused ACTIVATE (PSUM -> SBUF) per [128, 2048] tile.
  - dtypes: layer 0 runs bf16 (x and W quantization error here is
    contracted by ~0.7^10 through the remaining layers); layers 1-9 run
    float32r (ACT rounds its tanh outputs to fp32r so the PE takes the
    single-pass fast path instead of the 2-pass LOW_HIGH fp32 mode).
  - The final 2->1 linear is done on DVE: stream-transpose layer-10's
    activations back to channel-interleaved layout, then one dual-scalar
    tensor_scalar + one scalar_tensor_tensor (W_out/b_out baked as
    immediates) produce the point-major output for a contiguous store.
    This keeps PSUM free for the layer pipeline (2 x 4-bank slots
    double-buffered), so consecutive tile-pairs overlap without stalls.
  - ScalarE (tanh @ 1 elem/lane/cycle, 20 evals/point) is the roofline;
    macro-tiles are processed in pairs so ACT on tile A overlaps PE
    matmuls on tile B, keeping ACT ~100% busy.
"""

import numpy as np

N = 8388608
L = 10
NCORES = 8
NSHARD = N // NCORES  # 1048576
E = 2048  # raw tile free width (f32 per partition)
PTS_PER_TILE = 64 * E  # 131072
NTILES = NSHARD // PTS_PER_TILE  # 8

_CACHE = {}


def _build_program(w_out0, w_out1, b_out0):
    from contextlib import ExitStack
    from concourse import bacc, mybir, tile

    F32 = mybir.dt.float32
    F32R = mybir.dt.float32r
    BF16 = mybir.dt.bfloat16
    TANH = mybir.ActivationFunctionType.Tanh
    MULT = mybir.AluOpType.mult
    ADD = mybir.AluOpType.add

    zf = np.linspace(-0.6, 0.6, 40001)
    Af = np.stack([zf, zf**3, zf**5], 1)
    c0, c1, c2 = (float(v) for v in np.linalg.lstsq(Af, np.tanh(zf), rcond=None)[0])

    nc = bacc.Bacc(
        "TRN2",
        target_bir_lowering=False,
        debug=False,
        enable_asserts=False,
        num_devices=NCORES,
    )

    x_d = nc.dram_tensor("x", [NTILES, 128, E], F32, kind="ExternalInput").ap()
    wblk_d = nc.dram_tensor("wblk", [L, 128, 128], F32R, kind="ExternalInput").ap()
    wblk0_d = nc.dram_tensor("wblk0", [128, 128], BF16, kind="ExternalInput").ap()
    bias_d = nc.dram_tensor("biascols", [128, L], F32, kind="ExternalInput").ap()
    y_d = nc.dram_tensor("y", [NTILES, 128, E // 2], F32, kind="ExternalOutput").ap()

    with tile.TileContext(nc) as tc, ExitStack() as ctx:
        const_pool = ctx.enter_context(tc.tile_pool(name="const", bufs=1))
        raw_pool = ctx.enter_context(tc.tile_pool(name="raw", bufs=2))
        h_pool = ctx.enter_context(tc.tile_pool(name="h", bufs=6))
        psum_pool = ctx.enter_context(tc.tile_pool(name="ps", bufs=2, space="PSUM"))
        o2_pool = ctx.enter_context(tc.tile_pool(name="o2", bufs=2))
        o3_pool = ctx.enter_context(tc.tile_pool(name="o3", bufs=2))
        poly_pool = ctx.enter_context(tc.tile_pool(name="poly", bufs=1))

        w_tile = const_pool.tile([128, L * 128], F32R, tag="w")
        w0_tile = const_pool.tile([128, 128], BF16, tag="w0")
        bias_tile = const_pool.tile([128, L], F32, tag="b")

        # bias + layer-0 weights are needed by the very first ACT/matmul:
        # put them on the ACT HWDGE queue (idle until the first ACTIVATE)
        # so the sync queue starts the first x load immediately. The bulky
        # layer-1..9 weights (needed ~10us later) go on the gpsimd SWDGE
        # queue so they don't serialize ahead of the first x loads.
        nc.scalar.dma_start(bias_tile[:], bias_d[:])
        nc.scalar.dma_start(w0_tile[:], wblk0_d[:])
        for l in range(1, L):
            nc.gpsimd.dma_start(w_tile[:, 128 * l : 128 * (l + 1)], wblk_d[l])

        for pair in range(NTILES // 2):
            ts = (2 * pair, 2 * pair + 1)
            h = {}
            for t in ts:
                raw = raw_pool.tile([128, E], F32, tag="raw")
                xt = h_pool.tile([128, E], F32, tag="xin")
                xin = h_pool.tile([128, E], BF16, tag="xinb")
                if pair == 0:
                    # chunk the cold-start load so transpose/cast/matmul
                    # overlap the first DMA instead of waiting for 1MB
                    for c in range(4):
                        cs, ce = c * (E // 4), (c + 1) * (E // 4)
                        nc.sync.dma_start(raw[:, cs:ce], x_d[t][:, cs:ce])
                        nc.vector.transpose(xt[:, cs:ce], raw[:, cs:ce])
                        nc.vector.tensor_copy(xin[:, cs:ce], xt[:, cs:ce])
                else:
                    nc.sync.dma_start(raw[:], x_d[t])
                    nc.vector.transpose(xt[:], raw[:])
                    nc.vector.tensor_copy(xin[:], xt[:])
                h[t] = xin

            OFFLOAD = {(3, 6), (4, 6), (6, 6)}
            for l in range(L):
                for t in ts:
                    ps = psum_pool.tile([128, E], F32, tag="ps")
                    for j in range(E // 512):
                        nc.tensor.matmul(
                            ps[:, 512 * j : 512 * (j + 1)],
                            w0_tile[:] if l == 0 else w_tile[:, 128 * l : 128 * (l + 1)],
                            h[t][:, 512 * j : 512 * (j + 1)],
                            start=True,
                            stop=True,
                        )
                    hn = h_pool.tile([128, E], F32 if l == L - 1 else F32R, tag="h")
                    bias_ap = bias_tile[:, l : l + 1]
                    if (t, l) in OFFLOAD:
                        # tanh via deg-5 odd polynomial on DVE (|z| < 0.4 at
                        # this depth) to relieve the ACT bottleneck
                        HC = E // 2
                        for cc in range(2):
                            sl = slice(cc * HC, (cc + 1) * HC)
                            zs = poly_pool.tile([128, HC], F32, tag="zs", bufs=2)
                            nc.vector.tensor_scalar_add(zs[:], ps[:, sl], bias_ap)
                            u = poly_pool.tile([128, HC], F32, tag="u", bufs=2)
                            nc.vector.tensor_mul(u[:], zs[:], zs[:])
                            a = poly_pool.tile([128, HC], F32, tag="a", bufs=2)
                            nc.vector.tensor_scalar(a[:], u[:], c2, c1, MULT, ADD)
                            b = poly_pool.tile([128, HC], F32, tag="b", bufs=2)
                            nc.vector.scalar_tensor_tensor(b[:], a[:], 0.0, u[:], ADD, MULT)
                            nc.vector.scalar_tensor_tensor(hn[:, sl], b[:], c0, zs[:], ADD, MULT)
                    elif pair == 0 and l == 0:
                        # chunked so ACT starts as soon as the first matmul
                        # chunk lands during cold start
                        for j in range(E // 512):
                            sl = slice(512 * j, 512 * (j + 1))
                            nc.scalar.activation(
                                hn[:, sl], ps[:, sl], TANH, bias=bias_ap, scale=1.0
                            )
                    else:
                        nc.scalar.activation(hn[:], ps[:], TANH, bias=bias_ap, scale=1.0)
                    h[t] = hn

            for t in ts:
                # final linear on DVE, chunked so the last tile's store
                # starts as soon as the first chunk is ready
                o2 = o2_pool.tile([128, E], F32, tag="o2")
                o3 = o3_pool.tile([128, E // 2], F32, tag="o3")
                tmp = o3_pool.tile([128, E // 2], F32, tag="tmp")
                CH = 512
                for c in range(E // CH):
                    cs, ce = c * CH, (c + 1) * CH
                    ks, ke = cs // 2, ce // 2
                    nc.vector.transpose(o2[:, cs:ce], h[t][:, cs:ce])
                    o2v = o2[:, cs:ce].rearrange("p (k two) -> p k two", two=2)
                    # tmp = h1 * w_out1 + b_out ; o3 = (h0 * w_out0) + tmp
                    nc.vector.tensor_scalar(
                        tmp[:, ks:ke], o2v[:, :, 1], float(w_out1), float(b_out0), MULT, ADD
                    )
                    nc.vector.scalar_tensor_tensor(
                        o3[:, ks:ke], o2v[:, :, 0], float(w_out0), tmp[:, ks:ke], MULT, ADD
                    )
                    nc.sync.dma_start(y_d[t][:, ks:ke], o3[:, ks:ke])

    nc.compile()
    return nc


def _prep_consts(Ws, bs):
    import ml_dtypes

    wblk = np.zeros((L, 128, 128), np.float32)
    for l in range(L):
        WT = Ws[l].T.astype(np.float32)  # lhsT block = W_l^T
        for s in range(64):
            wblk[l, 2 * s : 2 * s + 2, 2 * s : 2 * s + 2] = WT
    wblk0 = wblk[0].astype(ml_dtypes.bfloat16)
    biascols = np.zeros((128, L), np.float32)
    biascols[0::2, :] = bs[:, 0][None, :]
    biascols[1::2, :] = bs[:, 1][None, :]
    return wblk, wblk0, biascols


def _run(x, Ws, bs, W_out, b_out, trace=False):
    from concourse.bass_utils import run_bass_kernel_spmd

    x = np.ascontiguousarray(np.asarray(x, dtype=np.float32))
    Ws = np.asarray(Ws, dtype=np.float32)
    bs = np.asarray(bs, dtype=np.float32)
    W_out = np.asarray(W_out, dtype=np.float32)
    b_out = np.asarray(b_out, dtype=np.float32)

    key = (float(W_out[0, 0]), float(W_out[0, 1]), float(b_out[0]))
    if _CACHE.get("key") != key:
        _CACHE["nc"] = _build_program(*key)
        _CACHE["key"] = key
    nc = _CACHE["nc"]

    wblk, wblk0, biascols = _prep_consts(Ws, bs)
    x_sh = x.reshape(NCORES, NTILES, 128, E)

    in_maps = [
        {
            "x": x_sh[c],
            "wblk": wblk,
            "wblk0": wblk0,
            "biascols": biascols,
        }
        for c in range(NCORES)
    ]
    res = run_bass_kernel_spmd(nc, in_maps, list(range(NCORES)), trace=trace)
    y = np.stack([res.results[c]["y"] for c in range(NCORES)])
    out = y.reshape(N, 1)
    return out, res


def kernel(x, Ws, bs, W_out, b_out):
    out, _ = _run(x, Ws, bs, W_out, b_out, trace=False)
    return out


# revision 17
# speedup vs baseline: 1.0111x; 1.0111x over previous
"""Trainium2 Bass kernel for a 10-layer tiny MLP over 8.4M 2-D points.

reference: h <- tanh(h @ W_l^T + b_l) for 10 layers (2->2), then y = h @ W_out^T + b_out.

Strategy (8 NeuronCores, pure data parallel over the batch):
  - Each core gets 1,048,576 points, processed as 8 macro-tiles of
    131,072 points ([128 partitions x 2048 f32] SBUF tiles).
  - Layout trick: a DVE 32x32 stream-transpose of the contiguously-loaded
    raw tile puts the two channels of each point on adjacent partitions
    (2s, 2s+1) of the same column (in a bit-shuffled point order, which is
    fine since the whole network is pointwise).
  - Each layer's affine transform is ONE 128x128 matmul per 512-col chunk
    with a block-diagonal weight matrix (64 copies of W_l^T on the
    diagonal), accumulated in PSUM; ScalarE applies bias+tanh in a single
    fused ACTIVATE (PSUM -> SBUF) per [128, 2048] tile.
  - dtypes: layer 0 runs bf16 (x and W quantization error here is
    contracted by ~0.7^10 through the remaining layers); layers 1-9 run
    float32r (ACT rounds its tanh outputs to fp32r so the PE takes the
    single-pass fast path instead of the 2-pass LOW_HIGH fp32 mode).
  - The final 2->1 linear is done on DVE: stream-transpose layer-10's
    activations back to channel-interleaved layout, then one dual-scalar
    tensor_scalar + one scalar_tensor_tensor (W_out/b_out baked as
    immediates) produce the point-major output for a contiguous store.
    This keeps PSUM free for the layer pipeline (2 x 4-bank slots
    double-buffered), so consecutive tile-pairs overlap without stalls.
  - ScalarE (tanh @ 1 elem/lane/cycle, 20 evals/point) is the roofline;
    macro-tiles are processed in pairs so ACT on tile A overlaps PE
    matmuls on tile B, keeping ACT ~100% busy.
"""

import numpy as np

N = 8388608
L = 10
NCORES = 8
NSHARD = N // NCORES  # 1048576
E = 2048  # raw tile free width (f32 per partition)
PTS_PER_TILE = 64 * E  # 131072
NTILES = NSHARD // PTS_PER_TILE  # 8

_CACHE = {}


def _build_program(w_out0, w_out1, b_out0):
    from contextlib import ExitStack
    from concourse import bacc, mybir, tile

    F32 = mybir.dt.float32
    F32R = mybir.dt.float32r
    BF16 = mybir.dt.bfloat16
    TANH = mybir.ActivationFunctionType.Tanh
    MULT = mybir.AluOpType.mult
    ADD = mybir.AluOpType.add

    zf = np.linspace(-0.75, 0.75, 40001)
    Af = np.stack([zf, zf**3, zf**5, zf**7], 1)
    c0, c1, c2, c3 = (float(v) for v in np.linalg.lstsq(Af, np.tanh(zf), rcond=None)[0])

    nc = bacc.Bacc(
        "TRN2",
        target_bir_lowering=False,
        debug=False,
        enable_asserts=False,
        num_devices=NCORES,
    )

    x_d = nc.dram_tensor("x", [NTILES, 128, E], F32, kind="ExternalInput").ap()
    wblk_d = nc.dram_tensor("wblk", [L, 128, 128], F32R, kind="ExternalInput").ap()
    wblk0_d = nc.dram_tensor("wblk0", [128, 128], BF16, kind="ExternalInput").ap()
    bias_d = nc.dram_tensor("biascols", [128, L], F32, kind="ExternalInput").ap()
    y_d = nc.dram_tensor("y", [NTILES, 128, E // 2], F32, kind="ExternalOutput").ap()

    with tile.TileContext(nc) as tc, ExitStack() as ctx:
        const_pool = ctx.enter_context(tc.tile_pool(name="const", bufs=1))
        raw_pool = ctx.enter_context(tc.tile_pool(name="raw", bufs=2))
        h_pool = ctx.enter_context(tc.tile_pool(name="h", bufs=6))
        psum_pool = ctx.enter_context(tc.tile_pool(name="ps", bufs=2, space="PSUM"))
        o2_pool = ctx.enter_context(tc.tile_pool(name="o2", bufs=2))
        o3_pool = ctx.enter_context(tc.tile_pool(name="o3", bufs=2))
        poly_pool = ctx.enter_context(tc.tile_pool(name="poly", bufs=1))

        w_tile = const_pool.tile([128, L * 128], F32R, tag="w")
        w0_tile = const_pool.tile([128, 128], BF16, tag="w0")
        bias_tile = const_pool.tile([128, L], F32, tag="b")

        # bias + layer-0 weights are needed by the very first ACT/matmul:
        # put them on the ACT HWDGE queue (idle until the first ACTIVATE)
        # so the sync queue starts the first x load immediately. The bulky
        # layer-1..9 weights (needed ~10us later) go on the gpsimd SWDGE
        # queue so they don't serialize ahead of the first x loads.
        nc.scalar.dma_start(bias_tile[:], bias_d[:])
        nc.scalar.dma_start(w0_tile[:], wblk0_d[:])
        for l in range(1, L):
            nc.gpsimd.dma_start(w_tile[:, 128 * l : 128 * (l + 1)], wblk_d[l])

        for pair in range(NTILES // 2):
            ts = (2 * pair, 2 * pair + 1)
            h = {}
            for t in ts:
                raw = raw_pool.tile([128, E], F32, tag="raw")
                xt = h_pool.tile([128, E], F32, tag="xin")
                xin = h_pool.tile([128, E], BF16, tag="xinb")
                if pair == 0:
                    # chunk the cold-start load so transpose/cast/matmul
                    # overlap the first DMA instead of waiting for 1MB
                    for c in range(4):
                        cs, ce = c * (E // 4), (c + 1) * (E // 4)
                        nc.sync.dma_start(raw[:, cs:ce], x_d[t][:, cs:ce])
                        nc.vector.transpose(xt[:, cs:ce], raw[:, cs:ce])
                        nc.vector.tensor_copy(xin[:, cs:ce], xt[:, cs:ce])
                else:
                    nc.sync.dma_start(raw[:], x_d[t])
                    nc.vector.transpose(xt[:], raw[:])
                    nc.vector.tensor_copy(xin[:], xt[:])
                h[t] = xin

            OFFLOAD = {(1, L - 1), (3, L - 1), (4, L - 1), (6, L - 1)}
            for l in range(L):
                for t in ts:
                    ps = psum_pool.tile([128, E], F32, tag="ps")
                    for j in range(E // 512):
                        nc.tensor.matmul(
                            ps[:, 512 * j : 512 * (j + 1)],
                            w0_tile[:] if l == 0 else w_tile[:, 128 * l : 128 * (l + 1)],
                            h[t][:, 512 * j : 512 * (j + 1)],
                            start=True,
                            stop=True,
                        )
                    hn = h_pool.tile([128, E], F32 if l == L - 1 else F32R, tag="h")
                    bias_ap = bias_tile[:, l : l + 1]
                    if (t, l) in OFFLOAD:
                        # tanh of the LAST layer via deg-7 odd polynomial on
                        # DVE (|z| < 0.4 here): its output feeds the DVE
                        # out-chain only, so this is off the ACT-critical
                        # layer chain entirely
                        HC = E // 2
                        for cc in range(2):
                            sl = slice(cc * HC, (cc + 1) * HC)
                            zs = poly_pool.tile([128, HC], F32, tag="zs", bufs=2)
                            nc.vector.tensor_scalar_add(zs[:], ps[:, sl], bias_ap)
                            u = poly_pool.tile([128, HC], F32, tag="u", bufs=2)
                            nc.vector.tensor_mul(u[:], zs[:], zs[:])
                            a = poly_pool.tile([128, HC], F32, tag="a", bufs=2)
                            nc.vector.tensor_scalar(a[:], u[:], c3, c2, MULT, ADD)
                            b = poly_pool.tile([128, HC], F32, tag="b", bufs=2)
                            nc.vector.scalar_tensor_tensor(b[:], a[:], 0.0, u[:], ADD, MULT)
                            nc.vector.scalar_tensor_tensor(b[:], b[:], c1, u[:], ADD, MULT)
                            nc.vector.scalar_tensor_tensor(hn[:, sl], b[:], c0, zs[:], ADD, MULT)
                    elif pair == 0 and l == 0:
                        # chunked so ACT starts as soon as the first matmul
                        # chunk lands during cold start
                        for j in range(E // 512):
                            sl = slice(512 * j, 512 * (j + 1))
                            nc.scalar.activation(
                                hn[:, sl], ps[:, sl], TANH, bias=bias_ap, scale=1.0
                            )
                    else:
                        nc.scalar.activation(hn[:], ps[:], TANH, bias=bias_ap, scale=1.0)
                    h[t] = hn

            for t in ts:
                # final linear on DVE, chunked so the last tile's store
                # starts as soon as the first chunk is ready
                o2 = o2_pool.tile([128, E], F32, tag="o2")
                o3 = o3_pool.tile([128, E // 2], F32, tag="o3")
                tmp = o3_pool.tile([128, E // 2], F32, tag="tmp")
                CH = 512
                for c in range(E // CH):
                    cs, ce = c * CH, (c + 1) * CH
                    ks, ke = cs // 2, ce // 2
                    nc.vector.transpose(o2[:, cs:ce], h[t][:, cs:ce])
                    o2v = o2[:, cs:ce].rearrange("p (k two) -> p k two", two=2)
                    # tmp = h1 * w_out1 + b_out ; o3 = (h0 * w_out0) + tmp
                    nc.vector.tensor_scalar(
                        tmp[:, ks:ke], o2v[:, :, 1], float(w_out1), float(b_out0), MULT, ADD
                    )
                    nc.vector.scalar_tensor_tensor(
                        o3[:, ks:ke], o2v[:, :, 0], float(w_out0), tmp[:, ks:ke], MULT, ADD
                    )
                    nc.sync.dma_start(y_d[t][:, ks:ke], o3[:, ks:ke])

    nc.compile()
    return nc


def _prep_consts(Ws, bs):
    import ml_dtypes

    wblk = np.zeros((L, 128, 128), np.float32)
    for l in range(L):
        WT = Ws[l].T.astype(np.float32)  # lhsT block = W_l^T
        for s in range(64):
            wblk[l, 2 * s : 2 * s + 2, 2 * s : 2 * s + 2] = WT
    wblk0 = wblk[0].astype(ml_dtypes.bfloat16)
    biascols = np.zeros((128, L), np.float32)
    biascols[0::2, :] = bs[:, 0][None, :]
    biascols[1::2, :] = bs[:, 1][None, :]
    return wblk, wblk0, biascols


def _run(x, Ws, bs, W_out, b_out, trace=False):
    from concourse.bass_utils import run_bass_kernel_spmd

    x = np.ascontiguousarray(np.asarray(x, dtype=np.float32))
    Ws = np.asarray(Ws, dtype=np.float32)
    bs = np.asarray(bs, dtype=np.float32)
    W_out = np.asarray(W_out, dtype=np.float32)
    b_out = np.asarray(b_out, dtype=np.float32)

    key = (float(W_out[0, 0]), float(W_out[0, 1]), float(b_out[0]))
    if _CACHE.get("key") != key:
        _CACHE["nc"] = _build_program(*key)
        _CACHE["key"] = key
    nc = _CACHE["nc"]

    wblk, wblk0, biascols = _prep_consts(Ws, bs)
    x_sh = x.reshape(NCORES, NTILES, 128, E)

    in_maps = [
        {
            "x": x_sh[c],
            "wblk": wblk,
            "wblk0": wblk0,
            "biascols": biascols,
        }
        for c in range(NCORES)
    ]
    res = run_bass_kernel_spmd(nc, in_maps, list(range(NCORES)), trace=trace)
    y = np.stack([res.results[c]["y"] for c in range(NCORES)])
    out = y.reshape(N, 1)
    return out, res


def kernel(x, Ws, bs, W_out, b_out):
    out, _ = _run(x, Ws, bs, W_out, b_out, trace=False)
    return out


# revision 20
# speedup vs baseline: 1.1042x; 1.0921x over previous
"""Trainium2 Bass kernel for a 10-layer tiny MLP over 8.4M 2-D points.

reference: h <- tanh(h @ W_l^T + b_l) for 10 layers (2->2), then y = h @ W_out^T + b_out.

Strategy (8 NeuronCores, pure data parallel over the batch):
  - Each core gets 1,048,576 points, processed as 8 macro-tiles of
    131,072 points ([128 partitions x 2048 f32] SBUF tiles).
  - Layout trick: a DVE 32x32 stream-transpose of the contiguously-loaded
    raw tile puts the two channels of each point on adjacent partitions
    (2s, 2s+1) of the same column (in a bit-shuffled point order, which is
    fine since the whole network is pointwise).
  - Each layer's affine transform is ONE 128x128 matmul per 512-col chunk
    with a block-diagonal weight matrix (64 copies of W_l^T on the
    diagonal), accumulated in PSUM; ScalarE applies bias+tanh in a single
    fused ACTIVATE (PSUM -> SBUF) per [128, 2048] tile.
  - dtypes: layer 0 runs bf16 (x and W quantization error here is
    contracted by ~0.7^10 through the remaining layers); layers 1-9 run
    float32r (ACT rounds its tanh outputs to fp32r so the PE takes the
    single-pass fast path instead of the 2-pass LOW_HIGH fp32 mode).
  - The final 2->1 linear is done on DVE: stream-transpose layer-10's
    activations back to channel-interleaved layout, then one dual-scalar
    tensor_scalar + one scalar_tensor_tensor (W_out/b_out baked as
    immediates) produce the point-major output for a contiguous store.
    This keeps PSUM free for the layer pipeline (2 x 4-bank slots
    double-buffered), so consecutive tile-pairs overlap without stalls.
  - ScalarE (tanh @ 1 elem/lane/cycle, 20 evals/point) is the roofline;
    macro-tiles are processed in pairs so ACT on tile A overlaps PE
    matmuls on tile B, keeping ACT ~100% busy.
"""

import numpy as np

N = 8388608
L = 10
NCORES = 8
NSHARD = N // NCORES  # 1048576
E = 2048  # raw tile free width (f32 per partition)
PTS_PER_TILE = 64 * E  # 131072
NTILES = NSHARD // PTS_PER_TILE  # 8

_CACHE = {}


def _build_program(w_out0, w_out1, b_out0):
    from contextlib import ExitStack
    from concourse import bacc, mybir, tile

    F32 = mybir.dt.float32
    F32R = mybir.dt.float32r
    BF16 = mybir.dt.bfloat16
    TANH = mybir.ActivationFunctionType.Tanh
    MULT = mybir.AluOpType.mult
    ADD = mybir.AluOpType.add

    zf = np.linspace(-0.75, 0.75, 40001)
    Af = np.stack([zf, zf**3, zf**5, zf**7], 1)
    c0, c1, c2, c3 = (float(v) for v in np.linalg.lstsq(Af, np.tanh(zf), rcond=None)[0])

    nc = bacc.Bacc(
        "TRN2",
        target_bir_lowering=False,
        debug=False,
        enable_asserts=False,
        num_devices=NCORES,
    )

    x_d = nc.dram_tensor("x", [NTILES, 128, E], F32, kind="ExternalInput").ap()
    wblk_d = nc.dram_tensor("wblk", [L, 128, 128], F32R, kind="ExternalInput").ap()
    wblk0_d = nc.dram_tensor("wblk0", [128, 128], BF16, kind="ExternalInput").ap()
    bias_d = nc.dram_tensor("biascols", [128, L], F32, kind="ExternalInput").ap()
    y_d = nc.dram_tensor("y", [NTILES, 128, E // 2], F32, kind="ExternalOutput").ap()

    with tile.TileContext(nc) as tc, ExitStack() as ctx:
        const_pool = ctx.enter_context(tc.tile_pool(name="const", bufs=1))
        raw_pool = ctx.enter_context(tc.tile_pool(name="raw", bufs=2))
        h_pool = ctx.enter_context(tc.tile_pool(name="h", bufs=6))
        psum_pool = ctx.enter_context(tc.tile_pool(name="ps", bufs=2, space="PSUM"))
        o2_pool = ctx.enter_context(tc.tile_pool(name="o2", bufs=2))
        o3_pool = ctx.enter_context(tc.tile_pool(name="o3", bufs=2))
        poly_pool = ctx.enter_context(tc.tile_pool(name="poly", bufs=1))

        w_tile = const_pool.tile([128, L * 128], F32R, tag="w")
        w0_tile = const_pool.tile([128, 128], BF16, tag="w0")
        bias_tile = const_pool.tile([128, L], F32, tag="b")

        # bias + layer-0 weights are needed by the very first ACT/matmul:
        # put them on the ACT HWDGE queue (idle until the first ACTIVATE)
        # so the sync queue starts the first x load immediately. The bulky
        # layer-1..9 weights (needed ~10us later) go on the gpsimd SWDGE
        # queue so they don't serialize ahead of the first x loads.
        nc.scalar.dma_start(bias_tile[:], bias_d[:])
        nc.scalar.dma_start(w0_tile[:], wblk0_d[:])
        for l in range(1, L):
            nc.gpsimd.dma_start(w_tile[:, 128 * l : 128 * (l + 1)], wblk_d[l])

        for pair in range(NTILES // 2):
            ts = (2 * pair, 2 * pair + 1)
            h = {}
            for t in ts:
                raw = raw_pool.tile([128, E], F32, tag="raw")
                xt = h_pool.tile([128, E], F32, tag="xin")
                xin = h_pool.tile([128, E], BF16, tag="xinb")
                if pair == 0:
                    # chunk the cold-start load so transpose/cast/matmul
                    # overlap the first DMA instead of waiting for 1MB
                    for c in range(4):
                        cs, ce = c * (E // 4), (c + 1) * (E // 4)
                        nc.sync.dma_start(raw[:, cs:ce], x_d[t][:, cs:ce])
                        nc.vector.transpose(xt[:, cs:ce], raw[:, cs:ce])
                        nc.gpsimd.tensor_copy(xin[:, cs:ce], xt[:, cs:ce])
                else:
                    nc.sync.dma_start(raw[:], x_d[t])
                    nc.vector.transpose(xt[:], raw[:])
                    nc.gpsimd.tensor_copy(xin[:], xt[:])
                h[t] = xin

            OFFLOAD = set()
            for l in range(L):
                for t in ts:
                    ps = psum_pool.tile([128, E], F32, tag="ps")
                    hn = h_pool.tile([128, E], F32 if l == L - 1 else F32R, tag="h")
                    bias_ap = bias_tile[:, l : l + 1]
                    if pair == 0 and l == 0 and (t, l) not in OFFLOAD:
                        # interleave matmul and ACT chunks during cold start
                        # so the first ACTIVATE waits on only one matmul
                        for j in range(E // 512):
                            sl = slice(512 * j, 512 * (j + 1))
                            nc.tensor.matmul(
                                ps[:, sl], w0_tile[:], h[t][:, sl], start=True, stop=True
                            )
                            nc.scalar.activation(
                                hn[:, sl], ps[:, sl], TANH, bias=bias_ap, scale=1.0
                            )
                        h[t] = hn
                        continue
                    for j in range(E // 512):
                        nc.tensor.matmul(
                            ps[:, 512 * j : 512 * (j + 1)],
                            w0_tile[:] if l == 0 else w_tile[:, 128 * l : 128 * (l + 1)],
                            h[t][:, 512 * j : 512 * (j + 1)],
                            start=True,
                            stop=True,
                        )
                    if (t, l) in OFFLOAD:
                        # tanh of the LAST layer via deg-7 odd polynomial on
                        # DVE (|z| < 0.4 here): its output feeds the DVE
                        # out-chain only, so this is off the ACT-critical
                        # layer chain entirely
                        HC = E // 2
                        for cc in range(2):
                            sl = slice(cc * HC, (cc + 1) * HC)
                            zs = poly_pool.tile([128, HC], F32, tag="zs", bufs=2)
                            nc.vector.tensor_scalar_add(zs[:], ps[:, sl], bias_ap)
                            u = poly_pool.tile([128, HC], F32, tag="u", bufs=2)
                            nc.vector.tensor_mul(u[:], zs[:], zs[:])
                            a = poly_pool.tile([128, HC], F32, tag="a", bufs=2)
                            nc.vector.tensor_scalar(a[:], u[:], c3, c2, MULT, ADD)
                            b = poly_pool.tile([128, HC], F32, tag="b", bufs=2)
                            nc.vector.scalar_tensor_tensor(b[:], a[:], 0.0, u[:], ADD, MULT)
                            nc.vector.scalar_tensor_tensor(b[:], b[:], c1, u[:], ADD, MULT)
                            nc.vector.scalar_tensor_tensor(hn[:, sl], b[:], c0, zs[:], ADD, MULT)
                    else:
                        nc.scalar.activation(hn[:], ps[:], TANH, bias=bias_ap, scale=1.0)
                    h[t] = hn

            for t in ts:
                # final linear on DVE, chunked so the last tile's store
                # starts as soon as the first chunk is ready
                o2 = o2_pool.tile([128, E], F32, tag="o2")
                o3 = o3_pool.tile([128, E // 2], F32, tag="o3")
                tmp = o3_pool.tile([128, E // 2], F32, tag="tmp")
                CH = 512
                for c in range(E // CH):
                    cs, ce = c * CH, (c + 1) * CH
                    ks, ke = cs // 2, ce // 2
                    nc.vector.transpose(o2[:, cs:ce], h[t][:, cs:ce])
                    o2v = o2[:, cs:ce].rearrange("p (k two) -> p k two", two=2)
                    # tmp = h1 * w_out1 + b_out ; o3 = (h0 * w_out0) + tmp
                    nc.vector.tensor_scalar(
                        tmp[:, ks:ke], o2v[:, :, 1], float(w_out1), float(b_out0), MULT, ADD
                    )
                    nc.vector.scalar_tensor_tensor(
                        o3[:, ks:ke], o2v[:, :, 0], float(w_out0), tmp[:, ks:ke], MULT, ADD
                    )
                    nc.sync.dma_start(y_d[t][:, ks:ke], o3[:, ks:ke])

    nc.compile()
    return nc


def _prep_consts(Ws, bs):
    import ml_dtypes

    wblk = np.zeros((L, 128, 128), np.float32)
    for l in range(L):
        WT = Ws[l].T.astype(np.float32)  # lhsT block = W_l^T
        for s in range(64):
            wblk[l, 2 * s : 2 * s + 2, 2 * s : 2 * s + 2] = WT
    wblk0 = wblk[0].astype(ml_dtypes.bfloat16)
    biascols = np.zeros((128, L), np.float32)
    biascols[0::2, :] = bs[:, 0][None, :]
    biascols[1::2, :] = bs[:, 1][None, :]
    return wblk, wblk0, biascols


def _run(x, Ws, bs, W_out, b_out, trace=False):
    from concourse.bass_utils import run_bass_kernel_spmd

    x = np.ascontiguousarray(np.asarray(x, dtype=np.float32))
    Ws = np.asarray(Ws, dtype=np.float32)
    bs = np.asarray(bs, dtype=np.float32)
    W_out = np.asarray(W_out, dtype=np.float32)
    b_out = np.asarray(b_out, dtype=np.float32)

    key = (float(W_out[0, 0]), float(W_out[0, 1]), float(b_out[0]))
    if _CACHE.get("key") != key:
        _CACHE["nc"] = _build_program(*key)
        _CACHE["key"] = key
    nc = _CACHE["nc"]

    wblk, wblk0, biascols = _prep_consts(Ws, bs)
    x_sh = x.reshape(NCORES, NTILES, 128, E)

    in_maps = [
        {
            "x": x_sh[c],
            "wblk": wblk,
            "wblk0": wblk0,
            "biascols": biascols,
        }
        for c in range(NCORES)
    ]
    res = run_bass_kernel_spmd(nc, in_maps, list(range(NCORES)), trace=trace)
    y = np.stack([res.results[c]["y"] for c in range(NCORES)])
    out = y.reshape(N, 1)
    return out, res


def kernel(x, Ws, bs, W_out, b_out):
    out, _ = _run(x, Ws, bs, W_out, b_out, trace=False)
    return out
